# revision 33
# baseline (speedup 1.0000x reference)
"""MLA (multi-head latent attention) forward on 8 TRN2 NeuronCores.

Sharding: 2-way data-parallel over batch x 4-way tensor-parallel over heads.
Core c handles batch b=c//4 and heads 4g..4g+3 where g=c%4. The host sums the
4 partial outputs per batch (the o_proj contribution of each head group).

Q path: RMSNorm is a per-token scalar, so it commutes with the B-projection:
q = rstd(x@q_a) * (x @ (q_a@q_b)). The folded W_q = q_a@q_b is precomputed on
the host and sharded by head (768 cols/core), killing the replicated q_a
matmul. rstd needs ||x@q_a||^2 over the full 1536-dim lora space; each core
computes a 384-col shard of it.

KV path: the 512-dim latent projection is column-sharded 4 ways (128 cols per
core) and AllGathered (512KB/rank) across the batch group; the rope slice is
per-head so it stays local. Both norm sum-of-squares partial vectors ride one
16KB AllReduce. All collectives overlap the folded-Wq stage.

Layout: activations are feature-major ([feature, token]) so every matmul
contracts over the partition dim; x arrives pre-transposed/pre-tiled from the
host. Scores are computed transposed (s[tk, tq]) so softmax needs no
max-subtraction (scores are bounded ~6) and P@V contracts naturally. The
causal diagonal blocks are processed triangularly (only query cols >= key
tile). Softmax denominators are accumulated on VectorE from the exp tiles and
turned into per-query sums by a single ones-matmul per (chunk, head). Rope q
is kept pair-packed in 64 partitions and contracted with a 64-partition
matmul, so no zero-padding is needed. RMSNorm scaling is applied at copy-out
of the projected tensors. Weights are pre-tiled on the host so every weight
DMA is contiguous.
"""

import numpy as np
import ml_dtypes

B, T, HIDDEN = 2, 2048, 2048
NUM_HEADS = 16
QK_NOPE, QK_ROPE, HEAD_DIM, V_HEAD = 128, 64, 192, 128
KV_LORA, Q_LORA = 512, 1536
EPS = 1e-6
NCORES = 8
HPC = 4  # heads per core

KC = HIDDEN // 128
TT = T // 128
TQ = T // 512
NS = Q_LORA // 4 // 128  # norm-shard col-tiles (3)
NL = KV_LORA // 128
NR = HPC * QK_ROPE // 128  # rope col-tiles (2)

BF16 = ml_dtypes.bfloat16
FP8 = ml_dtypes.float8_e4m3fn

_CACHE = {}


def _build():
    import concourse.bass as bass
    import concourse.tile as tile
    from concourse import bacc, mybir
    from concourse.bass import ts

    f32 = mybir.dt.float32
    bf = mybir.dt.bfloat16
    AF = mybir.ActivationFunctionType
    GROUPS = [[0, 1, 2, 3], [4, 5, 6, 7]]

    nc = bacc.Bacc(
        "TRN2",
        target_bir_lowering=False,
        debug=False,
        enable_asserts=True,
        num_devices=NCORES,
    )

    def din(name, shape, dt=bf):
        return nc.dram_tensor(name, shape, dt, kind="ExternalInput").ap()

    # weights pre-tiled on host: contiguous per-tile DMA loads
    x_ap = din("x", [TQ, 128, KC * 512])              # host-pretransposed rows
    qnw_ap = din("qnw", [NS, 128, KC, 128])           # norm-shard of q_a
    wq_ap = din("wq", [2, 128, KC, 2 * HEAD_DIM])     # folded q_a@q_b, pair:
    kvaw_ap = din("kvaw", [1 + NR, 128, KC, 128])     # [lat shard | ropes]
    kvbw_ap = din("kvbw", [128, NL, HPC * (QK_NOPE + V_HEAD)])
    ow_ap = din("ow", [128, HPC, HIDDEN])
    mask_ap = din("mask", [128, 128])                 # 0/1 causal block (bf16)
    ones128_ap = din("ones128", [128, 128])
    out_ap = nc.dram_tensor("out", [HIDDEN, T], bf, kind="ExternalOutput").ap()

    def eng(idx):
        return nc.scalar if idx % 2 else nc.vector

    def copy(e, out, in_):
        if e is nc.scalar:
            nc.scalar.copy(out, in_)
        else:
            nc.vector.tensor_copy(out, in_)

    with tile.TileContext(nc) as tc:
        with tc.tile_pool(name="consts", bufs=1) as consts, \
             tc.tile_pool(name="trans", bufs=3) as trans, \
             tc.tile_pool(name="dram", bufs=1, space="DRAM") as dram, \
             tc.tile_pool(name="act", bufs=1) as act, \
             tc.tile_pool(name="owp", bufs=1) as powp:

            mask = consts.tile([128, 128], bf)
            nc.sync.dma_start(out=mask, in_=mask_ap)
            ones128 = consts.tile([128, 128], bf)
            nc.sync.dma_start(out=ones128, in_=ones128_ap)
            eps1 = consts.tile([128, 1], f32)
            nc.vector.memset(eps1, EPS)
            ones1f = consts.tile([1, 128], f32)
            nc.vector.memset(ones1f, 1.0)
            onesf = consts.tile([128, 128], f32)
            nc.vector.memset(onesf, 1.0)

            ropes = act.tile([128, NR, T], bf)   # raw rope keys (2 heads/tile)
            rq_b = act.tile([128, T], f32)
            rkv_b = act.tile([128, T], f32)
            rkvT = act.tile([128, TT], f32)
            qn = act.tile([128, HPC, T], bf)
            qrp = act.tile([128, 2, T], bf)      # pair-packed rope q

            ar_in = dram.tile([1, 2 * T], f32)
            ar_out = dram.tile([1, 2 * T], f32)
            ag_in = dram.tile([128, T], bf)
            ag_out = dram.tile([NL, 128, T], bf)

            # ---- Stages N/KV/QF need xTq alive; freed before attention
            with tc.tile_pool(name="stageA", bufs=1) as pA, \
                 tc.tile_pool(name="wa", bufs=4) as pwa, \
                 tc.tile_pool(name="wD", bufs=1) as pw, \
                 tc.tile_pool(name="pB", bufs=1) as pB, \
                 tc.tile_pool(name="psumA", bufs=1, space="PSUM") as psumA:
                # x loads: quarter 0 first on both queues so stage N starts
                # early; qnw rides the scalar queue after quarter 0
                # head loads, priority order per DMA queue. x8 feeds stage
                # N (first compute); its quarters 2/3 reuse buffers, so they
                # go at the queue tails to avoid blocking xq/wq behind them.
                def load_xq(quarter):
                    xt = pA.tile([128, KC, 512], bf, name=f"xT{quarter}")
                    for g in range(4):
                        (nc.sync if g % 2 == 0 else nc.scalar).dma_start(
                            out=xt[:, 4 * g:4 * g + 4, :],
                            in_=x_ap[quarter, :, ts(g, 4 * 512)],
                        )
                    return xt

                qnw_tiles = []
                for n in range(NS):
                    wa = pwa.tile([128, KC, 128], bf, tag="wa", bufs=3)
                    nc.gpsimd.dma_start(out=wa, in_=qnw_ap[n])
                    qnw_tiles.append(wa)
                xTq = [load_xq(q) for q in range(TQ)]
                # kv weights on gpsimd (behind qnw); kvbw + one wq on
                # scalar, the other wq on sync, to balance the ~14.5MB of
                # front-phase traffic across the three ~80GB/s DMA queues
                kva_tiles = []
                for n in range(1 + NR):
                    wa = pwa.tile([128, KC, 128], bf, tag="wa", bufs=3)
                    nc.gpsimd.dma_start(out=wa, in_=kvaw_ap[n])
                    kva_tiles.append(wa)
                kvbw = powp.tile([128, NL, HPC * (QK_NOPE + V_HEAD)], bf)
                nc.scalar.dma_start(out=kvbw, in_=kvbw_ap)
                wq_tiles = []
                for pair in range(2):
                    wq = pw.tile([128, KC, 2 * HEAD_DIM], bf, tag="wq",
                                 bufs=2, name=f"wq{pair}")
                    (nc.sync if pair == 0 else nc.scalar).dma_start(
                        out=wq, in_=wq_ap[pair])
                    wq_tiles.append(wq)

                # ~40 back-to-back dummy matmuls warm the PE clock (HAM)
                # while the first x/weight DMAs stream in
                warm = psumA.tile([128, 128], f32, tag="warm", bufs=1)
                for w in range(40):
                    nc.tensor.matmul(out=warm, lhsT=ones128, rhs=ones128,
                                     start=(w == 0), stop=(w == 39))

                # ---- Stage N: 384-col shard of y=x@q_a, squared, summed
                # into per-token partials (partition-reduce via ones-matmul)
                for t in range(TQ):
                    psd = psumA.tile([128, 512], f32, tag="psd", bufs=2)
                    for n in range(NS):
                        psm = psumA.tile([128, 512], f32, tag="psm", bufs=4)
                        for kk in range(KC):
                            nc.tensor.matmul(
                                out=psm,
                                lhsT=qnw_tiles[n][:, kk, :],
                                rhs=xTq[t][:, kk, :],
                                start=(kk == 0),
                                stop=(kk == KC - 1),
                            )
                        sq = pB.tile([128, 512], bf, tag="sq", bufs=2)
                        nc.scalar.activation(out=sq, in_=psm, func=AF.Square)
                        nc.tensor.matmul(
                            out=psd, lhsT=ones128, rhs=sq,
                            start=(n == 0), stop=(n == NS - 1),
                        )
                    sst = pB.tile([1, 512], f32, tag="sst", bufs=2)
                    nc.scalar.copy(sst, psd[0:1, :])
                    nc.gpsimd.dma_start(ar_in[0:1, ts(t, 512)], sst)

                # ---- Stage KV (latent shard first): projection + sumsq
                for t in range(TQ):
                    psd = psumA.tile([128, 512], f32, tag="psd", bufs=2)
                    psm = psumA.tile([128, 512], f32, tag="psm", bufs=4)
                    for kk in range(KC):
                        nc.tensor.matmul(
                            out=psm,
                            lhsT=kva_tiles[0][:, kk, :],
                            rhs=xTq[t][:, kk, :],
                            start=(kk == 0),
                            stop=(kk == KC - 1),
                        )
                    lst = pB.tile([128, 512], bf, tag="lst", bufs=2)
                    nc.scalar.copy(lst, psm)
                    nc.gpsimd.dma_start(ag_in[:, ts(t, 512)], lst)
                    sq = pB.tile([128, 512], bf, tag="sq", bufs=2)
                    nc.scalar.activation(out=sq, in_=psm, func=AF.Square)
                    nc.tensor.matmul(out=psd, lhsT=ones128, rhs=sq,
                                     start=True, stop=True)
                    sst = pB.tile([1, 512], f32, tag="sst", bufs=2)
                    nc.scalar.copy(sst, psd[0:1, :])
                    nc.gpsimd.dma_start(
                        ar_in[0:1, T + t * 512:T + t * 512 + 512], sst)

                # one AllReduce carries both norm partial vectors; it and
                # the latent AllGather overlap the rope pass + folded-Wq
                nc.gpsimd.collective_compute(
                    "AllReduce",
                    mybir.AluOpType.add,
                    replica_groups=GROUPS,
                    ins=[ar_in.opt()],
                    outs=[ar_out.opt()],
                )
                nc.gpsimd.collective_compute(
                    "AllGather",
                    mybir.AluOpType.bypass,
                    replica_groups=GROUPS,
                    ins=[ag_in.opt()],
                    outs=[ag_out.opt()],
                )

                # ---- Stage KV (rope slices)
                for t in range(TQ):
                    for n in range(1, 1 + NR):
                        psm = psumA.tile([128, 512], f32, tag="psm", bufs=4)
                        for kk in range(KC):
                            nc.tensor.matmul(
                                out=psm,
                                lhsT=kva_tiles[n][:, kk, :],
                                rhs=xTq[t][:, kk, :],
                                start=(kk == 0),
                                stop=(kk == KC - 1),
                            )
                        nc.scalar.copy(ropes[:, n - 1, ts(t, 512)], psm)

                # ---- finish both rstds from the AllReduced sumsq: broadcast
                # raw sums via 1-row ones-matmul, then sqrt+recip full-width
                # (readback DMAs ride the scalar queue; gpsimd is blocked on
                # collective triggers)
                for half, (lora, rb) in enumerate(
                        ((Q_LORA, rq_b), (KV_LORA, rkv_b))):
                    for t in range(TQ):
                        rs1 = pB.tile([1, 512], f32, tag="sst", bufs=2)
                        nc.scalar.dma_start(
                            rs1[:], ar_out[0:1, half * T + t * 512:
                                           half * T + t * 512 + 512])
                        psb = psumA.tile([128, 512], f32, tag="psd", bufs=2)
                        nc.tensor.matmul(
                            out=psb, lhsT=ones1f, rhs=rs1,
                            start=True, stop=True,
                        )
                        tmp = pB.tile([128, 512], f32, tag="tmp", bufs=1)
                        nc.scalar.activation(
                            out=tmp, in_=psb, func=AF.Sqrt, bias=eps1,
                            scale=1.0 / lora,
                        )
                        nc.vector.reciprocal_approx_fast(
                            out=rb[:, ts(t, 512)], in_=tmp
                        )
                # transposed rstd_kv column view for the v row-scaling
                rkv_d = dram.tile([1, T], f32)
                nc.sync.dma_start(out=rkv_d, in_=rkv_b[0:1, :])
                nc.sync.dma_start(
                    out=rkvT,
                    in_=rkv_d.rearrange("o (tt p) -> (o p) tt", p=128),
                )

                # ---- Stage QF: q = rstd_q * (x @ W_q) per head pair
                for pair in range(2):
                    wq = wq_tiles[pair]
                    for t in range(TQ):
                        for sub in range(3):  # nope0 | nope1 | rope pair
                            ps = psumA.tile([128, 512], f32, tag="psm",
                                            bufs=4)
                            for kk in range(KC):
                                nc.tensor.matmul(
                                    out=ps,
                                    lhsT=wq[:, kk, ts(sub, 128)],
                                    rhs=xTq[t][:, kk, :],
                                    start=(kk == 0),
                                    stop=(kk == KC - 1),
                                )
                            if sub < 2:
                                nc.vector.tensor_mul(
                                    qn[:, 2 * pair + sub, ts(t, 512)],
                                    ps, rq_b[:, ts(t, 512)],
                                )
                            else:
                                nc.vector.tensor_mul(
                                    qrp[:, pair, ts(t, 512)],
                                    ps, rq_b[:, ts(t, 512)],
                                )

            # ---- Stages D/E/F
            with tc.tile_pool(name="att", bufs=1) as patt, \
                 tc.tile_pool(name="psumD", bufs=1, space="PSUM") as psumD:
                kn = patt.tile([128, HPC, T], bf)
                vv = patt.tile([128, TT, HPC * V_HEAD], bf)
                xlat = patt.tile([128, NL, T], bf)   # AllGathered raw latent
                for kk in range(NL):
                    nc.scalar.dma_start(out=xlat[:, kk, :], in_=ag_out[kk])
                ow = powp.tile([128, HPC, HIDDEN], bf)
                nc.gpsimd.dma_start(out=ow, in_=ow_ap)

                # ---- Stage D: k_nope (x rstd_kv), v (x rstd_kv)
                for h in range(HPC):
                    for t in range(TQ):
                        ps3 = psumD.tile([128, 512], f32, tag="psm", bufs=2)
                        for kk in range(NL):
                            nc.tensor.matmul(
                                out=ps3,
                                lhsT=kvbw[:, kk, ts(h, 256)][:, 0:128],
                                rhs=xlat[:, kk, ts(t, 512)],
                                start=(kk == 0),
                                stop=(kk == NL - 1),
                            )
                        nc.vector.tensor_mul(
                            kn[:, h, ts(t, 512)], ps3, rkv_b[:, ts(t, 512)]
                        )
                vcols = kvbw.rearrange(
                    "p kk (h two dv) -> p kk h two dv", h=HPC, two=2
                )
                for tt in range(TT):
                    psv = psumD.tile([128, 512], f32, tag="psm", bufs=2)
                    for kk in range(NL):
                        nc.tensor.matmul(
                            out=psv,
                            lhsT=xlat[:, kk, ts(tt, 128)],
                            rhs=vcols[:, kk, :, 1, :],
                            start=(kk == 0),
                            stop=(kk == NL - 1),
                        )
                    nc.scalar.mul(vv[:, tt, :], psv, mul=rkvT[:, tt:tt + 1])

                # ---- Stage E+F: causal attention; o_proj one chunk behind
                with tc.tile_pool(name="attn_i", bufs=2) as pai, \
                     tc.tile_pool(name="ob", bufs=2) as pob, \
                     tc.tile_pool(name="rdb", bufs=2) as prdb:
                    attn_tiles = {}

                    def attention_chunk(i):
                        attn_i = pai.tile([128, HPC, 512], bf, tag="attn_i", bufs=2)
                        for h in range(HPC):
                            nj = 4 * i + 4
                            pso = psumD.tile([128, 512], f32, tag="pso", bufs=2)
                            # exp-sum accumulated on VectorE; one ones-matmul
                            # per (i,h) turns it into the softmax denominator
                            acc = prdb.tile([128, 512], f32, tag="acc", bufs=2)

                            def consume_batch(batch, last):
                                for jc, exc, qb in batch:
                                    nc.tensor.matmul(
                                        out=pso[:, qb:512],
                                        lhsT=vv[:, jc, ts(h, V_HEAD)],
                                        rhs=exc[:, qb:512],
                                        start=(jc == 0),
                                        stop=(last and jc == batch[-1][0]),
                                    )

                            pending = []
                            for j in range(nj):
                                off = j * 128 - i * 512
                                qb = max(0, off)  # 1st query col this key sees
                                pss = psumD.tile([128, 512], f32, tag="pss", bufs=3)
                                nc.tensor.matmul(
                                    out=pss[:, qb:512],
                                    lhsT=kn[:, h, ts(j, 128)],
                                    rhs=qn[:, h, i * 512 + qb:(i + 1) * 512],
                                    start=True,
                                    stop=False,
                                )
                                hh = 64 * (h % 2)
                                nc.tensor.matmul(
                                    out=pss[:, qb:512],
                                    lhsT=ropes[hh:hh + 64, h // 2, ts(j, 128)],
                                    rhs=qrp[hh:hh + 64, h // 2,
                                            i * 512 + qb:(i + 1) * 512],
                                    start=False,
                                    stop=True,
                                )
                                if len(pending) == 4:
                                    consume_batch(pending, False)
                                    pending = []
                                ex = trans.tile([128, 512], bf, tag="ex", bufs=6)
                                nc.scalar.activation(
                                    out=ex[:, qb:512], in_=pss[:, qb:512],
                                    func=AF.Exp,
                                )
                                if off >= 0:
                                    nc.vector.tensor_mul(
                                        ex[:, qb:qb + 128], ex[:, qb:qb + 128],
                                        mask,
                                    )
                                if j == 0:
                                    nc.vector.tensor_copy(acc, ex)
                                else:
                                    nc.vector.tensor_add(
                                        acc[:, qb:512], acc[:, qb:512],
                                        ex[:, qb:512],
                                    )
                                pending.append((j, ex, qb))
                            if pending:
                                consume_batch(pending, True)

                            psd = psumD.tile([128, 512], f32, tag="psd", bufs=1)
                            nc.tensor.matmul(
                                out=psd, lhsT=onesf, rhs=acc,
                                start=True, stop=True,
                            )
                            rdb = prdb.tile([128, 512], f32, tag="rdb", bufs=2)
                            nc.vector.reciprocal_approx_fast(out=rdb, in_=psd)
                            nc.vector.tensor_mul(attn_i[:, h, :], pso, rdb)
                        attn_tiles[i] = attn_i

                    def oproj_chunk(i):
                        attn_i = attn_tiles[i]
                        for m in range(TT):
                            psf = psumD.tile([128, 512], f32, tag="psm", bufs=2)
                            for kk in range(HPC):
                                nc.tensor.matmul(
                                    out=psf,
                                    lhsT=ow[:, kk, ts(m, 128)],
                                    rhs=attn_i[:, kk, :],
                                    start=(kk == 0),
                                    stop=(kk == HPC - 1),
                                )
                            ob = pob.tile([128, 512], bf, tag="ob", bufs=3)
                            if i == TQ - 1:
                                copy(eng(m), ob, psf)  # attention done: ACT free
                            else:
                                nc.vector.tensor_copy(ob, psf)
                            (nc.sync if m % 2 else nc.gpsimd).dma_start(
                                out=out_ap[ts(m, 128), ts(i, 512)], in_=ob
                            )

                    # big chunks first: the tail then drains on the
                    # smallest attention chunk + its o_proj
                    order = list(range(TQ))[::-1]
                    attention_chunk(order[0])
                    for k in range(1, TQ):
                        attention_chunk(order[k])
                        oproj_chunk(order[k - 1])
                    oproj_chunk(order[-1])

    nc.compile()
    return nc


def _tile_w(w):
    """[K, N] -> [N/128, 128, K/128, 128] so each col-block loads contiguously."""
    K, N = w.shape
    return np.ascontiguousarray(
        w.reshape(K // 128, 128, N // 128, 128).transpose(2, 1, 0, 3))


def _prep(inputs):
    x = np.asarray(inputs["hidden_states"], np.float32)
    qaw = np.asarray(inputs["q_a_w"], np.float32)
    qalw = np.asarray(inputs["q_a_ln_w"], np.float32)
    qbw = np.asarray(inputs["q_b_w"], np.float32)
    kvaw = np.asarray(inputs["kv_a_w"], np.float32)
    kvlw = np.asarray(inputs["kv_a_ln_w"], np.float32)
    kvbw = np.asarray(inputs["kv_b_w"], np.float32)
    ow = np.asarray(inputs["o_w"], np.float32)

    scale = 1.0 / np.sqrt(np.float32(HEAD_DIM))
    qbw_f = qbw * qalw[:, None] * scale
    wq_full = qaw @ qbw_f  # [HIDDEN, NUM_HEADS*HEAD_DIM] fp32 fold
    kvbw_f = (kvbw * kvlw[:, None]).astype(BF16)

    r = np.arange(128)[:, None]
    j = np.arange(128)[None, :]
    mask = np.where(j >= r, 1.0, 0.0).astype(BF16)
    ones128 = np.ones((128, 128), BF16)

    def lat_tiled(w):  # [KV_LORA, N] -> [128, NL, N] (p, kk, n)
        return np.ascontiguousarray(w.reshape(NL, 128, -1).transpose(1, 0, 2))

    in_maps = []
    for c in range(NCORES):
        b, g = c // 4, c % 4
        # folded W_q for this head group, pair layout [nope0|nope1|ropes]
        wq_g = wq_full[:, g * HPC * HEAD_DIM:(g + 1) * HPC * HEAD_DIM]
        pairs = []
        for pair in range(HPC // 2):
            h0, h1 = 2 * pair, 2 * pair + 1
            cols = np.concatenate([
                wq_g[:, h0 * HEAD_DIM:h0 * HEAD_DIM + QK_NOPE],
                wq_g[:, h1 * HEAD_DIM:h1 * HEAD_DIM + QK_NOPE],
                wq_g[:, h0 * HEAD_DIM + QK_NOPE:(h0 + 1) * HEAD_DIM],
                wq_g[:, h1 * HEAD_DIM + QK_NOPE:(h1 + 1) * HEAD_DIM],
            ], axis=1).astype(BF16)  # [HIDDEN, 384]
            pairs.append(cols.reshape(KC, 128, 384).transpose(1, 0, 2))
        wq_c = np.ascontiguousarray(np.stack(pairs))   # [2, 128, KC, 384]

        # norm-shard of q_a: 384 cols per core
        qnw = _tile_w(np.ascontiguousarray(
            qaw[:, g * 384:(g + 1) * 384]).astype(BF16))

        # kv_a: this core's 128-col latent shard + its 4 heads' rope cols
        kvaw_g = np.concatenate(
            [kvaw[:, g * 128:(g + 1) * 128],
             kvaw[:, KV_LORA + g * HPC * QK_ROPE:
                  KV_LORA + (g + 1) * HPC * QK_ROPE]], axis=1).astype(BF16)

        xt = np.ascontiguousarray(
            x[b].T.astype(BF16).reshape(KC, 128, TQ, 512).transpose(2, 1, 0, 3)
            .reshape(TQ, 128, KC * 512))
        in_maps.append({
            "x": xt,
            "qnw": qnw,
            "wq": wq_c,
            "kvaw": _tile_w(kvaw_g),
            "kvbw": lat_tiled(kvbw_f[:, g * HPC * 256:(g + 1) * HPC * 256]),
            "ow": np.ascontiguousarray(
                ow[g * HPC * V_HEAD:(g + 1) * HPC * V_HEAD]
                .astype(BF16).reshape(HPC, 128, HIDDEN).transpose(1, 0, 2)),
            "mask": mask,
            "ones128": ones128,
        })
    return in_maps


def _ensure_trace_shim():
    """This image lacks antenv.axon_hooks; synthesize it so a trace=True (or
    BASS_TRACE=1) invocation degrades gracefully instead of crashing."""
    import sys
    import types
    try:
        import antenv.axon_hooks  # noqa: F401
        return
    except Exception:
        pass
    try:
        import antenv
        import trn_agent_boot.trn_boot as tb
        hook = tb._ntff_profile_via_ctypes("/opt/axon/libaxon_pjrt.so")
        mod = types.ModuleType("antenv.axon_hooks")
        mod.get_axon_ntff_profile_hook = lambda: hook
        mod.set_axon_ntff_profile_hook = lambda h: None
        antenv.axon_hooks = mod
        sys.modules["antenv.axon_hooks"] = mod
        import concourse.bass_utils as bu
        bu.upload_artifacts = lambda tmpdir: tmpdir
    except Exception:
        pass


def kernel(**inputs):
    from concourse.bass_utils import run_bass_kernel_spmd

    _ensure_trace_shim()
    if "nc" not in _CACHE:
        _CACHE["nc"] = _build()
    nc = _CACHE["nc"]
    in_maps = _prep(inputs)
    try:
        res = run_bass_kernel_spmd(nc, in_maps, core_ids=list(range(NCORES)),
                                   **_CACHE.get("run_kwargs", {}))
    except Exception:
        # transient accelerator faults (e.g. NRT_EXEC_UNIT_UNRECOVERABLE) have
        # been observed after interrupted runs; one retry clears them
        import time
        time.sleep(2)
        res = run_bass_kernel_spmd(nc, in_maps, core_ids=list(range(NCORES)),
                                   **_CACHE.get("run_kwargs", {}))
    _CACHE["last_results"] = res
    out = np.zeros((B, T, HIDDEN), np.float32)
    for c in range(NCORES):
        out[c // 4] += np.asarray(res.results[c]["out"], np.float32).T
    return out


# revision 34
# speedup vs baseline: 1.0290x; 1.0290x over previous
"""MLA (multi-head latent attention) forward on 8 TRN2 NeuronCores.

Sharding: 2-way data-parallel over batch x 4-way tensor-parallel over heads.
Core c handles batch b=c//4 and heads 4g..4g+3 where g=c%4. The host sums the
4 partial outputs per batch (the o_proj contribution of each head group).

Q path: RMSNorm is a per-token scalar, so it commutes with the B-projection:
q = rstd(x@q_a) * (x @ (q_a@q_b)). The folded W_q = q_a@q_b is precomputed on
the host and sharded by head (768 cols/core), killing the replicated q_a
matmul. rstd needs ||x@q_a||^2 over the full 1536-dim lora space; each core
computes a 384-col shard of it.

KV path: the 512-dim latent projection is column-sharded 4 ways (128 cols per
core) and AllGathered (512KB/rank) across the batch group; the rope slice is
per-head so it stays local. Both norm sum-of-squares partial vectors ride one
16KB AllReduce. All collectives overlap the folded-Wq stage.

Layout: activations are feature-major ([feature, token]) so every matmul
contracts over the partition dim; x arrives pre-transposed/pre-tiled from the
host. Scores are computed transposed (s[tk, tq]) so softmax needs no
max-subtraction (scores are bounded ~6) and P@V contracts naturally. The
causal diagonal blocks are processed triangularly (only query cols >= key
tile). Softmax denominators are accumulated on VectorE from the exp tiles and
turned into per-query sums by a single ones-matmul per (chunk, head). Rope q
is kept pair-packed in 64 partitions and contracted with a 64-partition
matmul, so no zero-padding is needed. RMSNorm scaling is applied at copy-out
of the projected tensors. Weights are pre-tiled on the host so every weight
DMA is contiguous.
"""

import numpy as np
import ml_dtypes

B, T, HIDDEN = 2, 2048, 2048
NUM_HEADS = 16
QK_NOPE, QK_ROPE, HEAD_DIM, V_HEAD = 128, 64, 192, 128
KV_LORA, Q_LORA = 512, 1536
EPS = 1e-6
NCORES = 8
HPC = 4  # heads per core

KC = HIDDEN // 128
TT = T // 128
TQ = T // 512
NS = Q_LORA // 4 // 128  # norm-shard col-tiles (3)
NL = KV_LORA // 128
NR = HPC * QK_ROPE // 128  # rope col-tiles (2)

BF16 = ml_dtypes.bfloat16
FP8 = ml_dtypes.float8_e4m3fn

_CACHE = {}


def _build():
    import concourse.bass as bass
    import concourse.tile as tile
    from concourse import bacc, mybir
    from concourse.bass import ts

    f32 = mybir.dt.float32
    bf = mybir.dt.bfloat16
    AF = mybir.ActivationFunctionType
    GROUPS = [[0, 1, 2, 3], [4, 5, 6, 7]]

    nc = bacc.Bacc(
        "TRN2",
        target_bir_lowering=False,
        debug=False,
        enable_asserts=True,
        num_devices=NCORES,
    )

    def din(name, shape, dt=bf):
        return nc.dram_tensor(name, shape, dt, kind="ExternalInput").ap()

    # weights pre-tiled on host: contiguous per-tile DMA loads
    x_ap = din("x", [TQ, 128, KC * 512])              # host-pretransposed rows
    qnw_ap = din("qnw", [NS, 128, KC, 128])           # norm-shard of q_a
    wq_ap = din("wq", [2, 128, KC, 2 * HEAD_DIM])     # folded q_a@q_b, pair:
    kvaw_ap = din("kvaw", [1 + NR, 128, KC, 128])     # [lat shard | ropes]
    kvbw_ap = din("kvbw", [128, NL, HPC * (QK_NOPE + V_HEAD)])
    ow_ap = din("ow", [128, HPC, HIDDEN])
    mask_ap = din("mask", [128, 128])                 # 0/1 causal block (bf16)
    ones128_ap = din("ones128", [128, 128])
    out_ap = nc.dram_tensor("out", [HIDDEN, T], bf, kind="ExternalOutput").ap()

    def eng(idx):
        return nc.scalar if idx % 2 else nc.vector

    def copy(e, out, in_):
        if e is nc.scalar:
            nc.scalar.copy(out, in_)
        else:
            nc.vector.tensor_copy(out, in_)

    with tile.TileContext(nc) as tc:
        with tc.tile_pool(name="consts", bufs=1) as consts, \
             tc.tile_pool(name="trans", bufs=3) as trans, \
             tc.tile_pool(name="dram", bufs=1, space="DRAM") as dram, \
             tc.tile_pool(name="act", bufs=1) as act, \
             tc.tile_pool(name="owp", bufs=1) as powp:

            mask = consts.tile([128, 128], bf)
            nc.sync.dma_start(out=mask, in_=mask_ap)
            ones128 = consts.tile([128, 128], bf)
            nc.sync.dma_start(out=ones128, in_=ones128_ap)
            eps1 = consts.tile([128, 1], f32)
            nc.vector.memset(eps1, EPS)
            ones1f = consts.tile([1, 128], f32)
            nc.vector.memset(ones1f, 1.0)
            onesf = consts.tile([128, 128], f32)
            nc.vector.memset(onesf, 1.0)

            ropes = act.tile([128, NR, T], bf)   # raw rope keys (2 heads/tile)
            rq_b = act.tile([128, T], f32)
            rkv_b = act.tile([128, T], f32)
            rkvT = act.tile([128, TT], f32)
            qn = act.tile([128, HPC, T], bf)
            qrp = act.tile([128, 2, T], bf)      # pair-packed rope q

            ar_in = dram.tile([1, 2 * T], f32)
            ar_out = dram.tile([1, 2 * T], f32)
            ag_in = dram.tile([128, T], bf)
            ag_out = dram.tile([NL, 128, T], bf)

            # ---- Stages N/KV/QF need xTq alive; freed before attention
            with tc.tile_pool(name="stageA", bufs=1) as pA, \
                 tc.tile_pool(name="wa", bufs=4) as pwa, \
                 tc.tile_pool(name="wD", bufs=1) as pw, \
                 tc.tile_pool(name="pB", bufs=1) as pB, \
                 tc.tile_pool(name="psumA", bufs=1, space="PSUM") as psumA:
                # x loads: quarter 0 first on both queues so stage N starts
                # early; qnw rides the scalar queue after quarter 0
                # head loads, priority order per DMA queue. x8 feeds stage
                # N (first compute); its quarters 2/3 reuse buffers, so they
                # go at the queue tails to avoid blocking xq/wq behind them.
                def load_xq(quarter):
                    xt = pA.tile([128, KC, 512], bf, name=f"xT{quarter}")
                    for g in range(4):
                        (nc.sync if g % 2 == 0 else nc.scalar).dma_start(
                            out=xt[:, 4 * g:4 * g + 4, :],
                            in_=x_ap[quarter, :, ts(g, 4 * 512)],
                        )
                    return xt

                qnw_tiles = []
                for n in range(NS):
                    wa = pwa.tile([128, KC, 128], bf, tag="wa", bufs=3)
                    nc.gpsimd.dma_start(out=wa, in_=qnw_ap[n])
                    qnw_tiles.append(wa)
                xTq = [load_xq(q) for q in range(TQ)]
                # kv weights on gpsimd (behind qnw); kvbw + one wq on
                # scalar, the other wq on sync, to balance the ~14.5MB of
                # front-phase traffic across the three ~80GB/s DMA queues
                kva_tiles = []
                for n in range(1 + NR):
                    wa = pwa.tile([128, KC, 128], bf, tag="wa", bufs=3)
                    nc.gpsimd.dma_start(out=wa, in_=kvaw_ap[n])
                    kva_tiles.append(wa)
                kvbw = powp.tile([128, NL, HPC * (QK_NOPE + V_HEAD)], bf)
                nc.scalar.dma_start(out=kvbw, in_=kvbw_ap)
                wq_tiles = []
                for pair in range(2):
                    wq = pw.tile([128, KC, 2 * HEAD_DIM], bf, tag="wq",
                                 bufs=2, name=f"wq{pair}")
                    (nc.sync if pair == 0 else nc.scalar).dma_start(
                        out=wq, in_=wq_ap[pair])
                    wq_tiles.append(wq)

                # ~40 back-to-back dummy matmuls warm the PE clock (HAM)
                # while the first x/weight DMAs stream in
                warm = psumA.tile([128, 128], f32, tag="warm", bufs=1)
                for w in range(40):
                    nc.tensor.matmul(out=warm, lhsT=ones128, rhs=ones128,
                                     start=(w == 0), stop=(w == 39))

                # ---- Stage N: 384-col shard of y=x@q_a, squared, summed
                # into per-token partials (partition-reduce via ones-matmul)
                for t in range(TQ):
                    psd = psumA.tile([128, 512], f32, tag="psd", bufs=2)
                    for n in range(NS):
                        psm = psumA.tile([128, 512], f32, tag="psm", bufs=4)
                        for kk in range(KC):
                            nc.tensor.matmul(
                                out=psm,
                                lhsT=qnw_tiles[n][:, kk, :],
                                rhs=xTq[t][:, kk, :],
                                start=(kk == 0),
                                stop=(kk == KC - 1),
                            )
                        sq = pB.tile([128, 512], bf, tag="sq", bufs=2)
                        nc.scalar.activation(out=sq, in_=psm, func=AF.Square)
                        nc.tensor.matmul(
                            out=psd, lhsT=ones128, rhs=sq,
                            start=(n == 0), stop=(n == NS - 1),
                        )
                    sst = pB.tile([1, 512], f32, tag="sst", bufs=2)
                    nc.scalar.copy(sst, psd[0:1, :])
                    nc.gpsimd.dma_start(ar_in[0:1, ts(t, 512)], sst)

                # ---- Stage KV (latent shard first): projection + sumsq
                for t in range(TQ):
                    psd = psumA.tile([128, 512], f32, tag="psd", bufs=2)
                    psm = psumA.tile([128, 512], f32, tag="psm", bufs=4)
                    for kk in range(KC):
                        nc.tensor.matmul(
                            out=psm,
                            lhsT=kva_tiles[0][:, kk, :],
                            rhs=xTq[t][:, kk, :],
                            start=(kk == 0),
                            stop=(kk == KC - 1),
                        )
                    lst = pB.tile([128, 512], bf, tag="lst", bufs=2)
                    nc.scalar.copy(lst, psm)
                    nc.gpsimd.dma_start(ag_in[:, ts(t, 512)], lst)
                    sq = pB.tile([128, 512], bf, tag="sq", bufs=2)
                    nc.scalar.activation(out=sq, in_=psm, func=AF.Square)
                    nc.tensor.matmul(out=psd, lhsT=ones128, rhs=sq,
                                     start=True, stop=True)
                    sst = pB.tile([1, 512], f32, tag="sst", bufs=2)
                    nc.scalar.copy(sst, psd[0:1, :])
                    nc.gpsimd.dma_start(
                        ar_in[0:1, T + t * 512:T + t * 512 + 512], sst)

                # one AllReduce carries both norm partial vectors; it and
                # the latent AllGather overlap the rope pass + folded-Wq
                nc.gpsimd.collective_compute(
                    "AllReduce",
                    mybir.AluOpType.add,
                    replica_groups=GROUPS,
                    ins=[ar_in.opt()],
                    outs=[ar_out.opt()],
                )
                nc.gpsimd.collective_compute(
                    "AllGather",
                    mybir.AluOpType.bypass,
                    replica_groups=GROUPS,
                    ins=[ag_in.opt()],
                    outs=[ag_out.opt()],
                )

                # ---- Stage KV (rope slices)
                for t in range(TQ):
                    for n in range(1, 1 + NR):
                        psm = psumA.tile([128, 512], f32, tag="psm", bufs=4)
                        for kk in range(KC):
                            nc.tensor.matmul(
                                out=psm,
                                lhsT=kva_tiles[n][:, kk, :],
                                rhs=xTq[t][:, kk, :],
                                start=(kk == 0),
                                stop=(kk == KC - 1),
                            )
                        nc.scalar.copy(ropes[:, n - 1, ts(t, 512)], psm)

                # ---- Stage QF: q = rstd_q * (x @ W_q) per head pair
                for pair in range(2):
                    wq = wq_tiles[pair]
                    for t in range(TQ):
                        for sub in range(3):  # nope0 | nope1 | rope pair
                            ps = psumA.tile([128, 512], f32, tag="psm",
                                            bufs=4)
                            for kk in range(KC):
                                nc.tensor.matmul(
                                    out=ps,
                                    lhsT=wq[:, kk, ts(sub, 128)],
                                    rhs=xTq[t][:, kk, :],
                                    start=(kk == 0),
                                    stop=(kk == KC - 1),
                                )
                            if sub < 2:
                                nc.scalar.copy(
                                    qn[:, 2 * pair + sub, ts(t, 512)], ps)
                            else:
                                nc.scalar.copy(
                                    qrp[:, pair, ts(t, 512)], ps)

                # ---- finish both rstds from the AllReduced sumsq (emitted
                # after QF so a slow collective never stalls the PE stream):
                # broadcast raw sums via 1-row ones-matmul, sqrt+recip, then
                # normalize q in place on VectorE
                for half, (lora, rb) in enumerate(
                        ((Q_LORA, rq_b), (KV_LORA, rkv_b))):
                    for t in range(TQ):
                        rs1 = pB.tile([1, 512], f32, tag="sst", bufs=2)
                        nc.scalar.dma_start(
                            rs1[:], ar_out[0:1, half * T + t * 512:
                                           half * T + t * 512 + 512])
                        psb = psumA.tile([128, 512], f32, tag="psd", bufs=2)
                        nc.tensor.matmul(
                            out=psb, lhsT=ones1f, rhs=rs1,
                            start=True, stop=True,
                        )
                        tmp = pB.tile([128, 512], f32, tag="tmp", bufs=1)
                        nc.scalar.activation(
                            out=tmp, in_=psb, func=AF.Sqrt, bias=eps1,
                            scale=1.0 / lora,
                        )
                        nc.vector.reciprocal_approx_fast(
                            out=rb[:, ts(t, 512)], in_=tmp
                        )
                # transposed rstd_kv column view for the v row-scaling
                rkv_d = dram.tile([1, T], f32)
                nc.sync.dma_start(out=rkv_d, in_=rkv_b[0:1, :])
                nc.sync.dma_start(
                    out=rkvT,
                    in_=rkv_d.rearrange("o (tt p) -> (o p) tt", p=128),
                )
                for t in range(TQ):
                    for h in range(HPC):
                        nc.vector.tensor_mul(
                            qn[:, h, ts(t, 512)], qn[:, h, ts(t, 512)],
                            rq_b[:, ts(t, 512)],
                        )
                    for pr in range(2):
                        nc.vector.tensor_mul(
                            qrp[:, pr, ts(t, 512)], qrp[:, pr, ts(t, 512)],
                            rq_b[:, ts(t, 512)],
                        )

            # ---- Stages D/E/F
            with tc.tile_pool(name="att", bufs=1) as patt, \
                 tc.tile_pool(name="psumD", bufs=1, space="PSUM") as psumD:
                kn = patt.tile([128, HPC, T], bf)
                vv = patt.tile([128, TT, HPC * V_HEAD], bf)
                xlat = patt.tile([128, NL, T], bf)   # AllGathered raw latent
                for kk in range(NL):
                    nc.scalar.dma_start(out=xlat[:, kk, :], in_=ag_out[kk])
                ow = powp.tile([128, HPC, HIDDEN], bf)
                nc.gpsimd.dma_start(out=ow, in_=ow_ap)

                # ---- Stage D: k_nope (x rstd_kv), v (x rstd_kv)
                for h in range(HPC):
                    for t in range(TQ):
                        ps3 = psumD.tile([128, 512], f32, tag="psm", bufs=2)
                        for kk in range(NL):
                            nc.tensor.matmul(
                                out=ps3,
                                lhsT=kvbw[:, kk, ts(h, 256)][:, 0:128],
                                rhs=xlat[:, kk, ts(t, 512)],
                                start=(kk == 0),
                                stop=(kk == NL - 1),
                            )
                        nc.vector.tensor_mul(
                            kn[:, h, ts(t, 512)], ps3, rkv_b[:, ts(t, 512)]
                        )
                vcols = kvbw.rearrange(
                    "p kk (h two dv) -> p kk h two dv", h=HPC, two=2
                )
                for tt in range(TT):
                    psv = psumD.tile([128, 512], f32, tag="psm", bufs=2)
                    for kk in range(NL):
                        nc.tensor.matmul(
                            out=psv,
                            lhsT=xlat[:, kk, ts(tt, 128)],
                            rhs=vcols[:, kk, :, 1, :],
                            start=(kk == 0),
                            stop=(kk == NL - 1),
                        )
                    nc.scalar.mul(vv[:, tt, :], psv, mul=rkvT[:, tt:tt + 1])

                # ---- Stage E+F: causal attention; o_proj one chunk behind
                with tc.tile_pool(name="attn_i", bufs=2) as pai, \
                     tc.tile_pool(name="ob", bufs=2) as pob, \
                     tc.tile_pool(name="rdb", bufs=2) as prdb:
                    attn_tiles = {}

                    def attention_chunk(i):
                        attn_i = pai.tile([128, HPC, 512], bf, tag="attn_i", bufs=2)
                        for h in range(HPC):
                            nj = 4 * i + 4
                            pso = psumD.tile([128, 512], f32, tag="pso", bufs=2)
                            # exp-sum accumulated on VectorE; one ones-matmul
                            # per (i,h) turns it into the softmax denominator
                            acc = prdb.tile([128, 512], f32, tag="acc", bufs=2)

                            def consume_batch(batch, last):
                                for jc, exc, qb in batch:
                                    nc.tensor.matmul(
                                        out=pso[:, qb:512],
                                        lhsT=vv[:, jc, ts(h, V_HEAD)],
                                        rhs=exc[:, qb:512],
                                        start=(jc == 0),
                                        stop=(last and jc == batch[-1][0]),
                                    )

                            pending = []
                            for j in range(nj):
                                off = j * 128 - i * 512
                                qb = max(0, off)  # 1st query col this key sees
                                pss = psumD.tile([128, 512], f32, tag="pss", bufs=3)
                                nc.tensor.matmul(
                                    out=pss[:, qb:512],
                                    lhsT=kn[:, h, ts(j, 128)],
                                    rhs=qn[:, h, i * 512 + qb:(i + 1) * 512],
                                    start=True,
                                    stop=False,
                                )
                                hh = 64 * (h % 2)
                                nc.tensor.matmul(
                                    out=pss[:, qb:512],
                                    lhsT=ropes[hh:hh + 64, h // 2, ts(j, 128)],
                                    rhs=qrp[hh:hh + 64, h // 2,
                                            i * 512 + qb:(i + 1) * 512],
                                    start=False,
                                    stop=True,
                                )
                                if len(pending) == 4:
                                    consume_batch(pending, False)
                                    pending = []
                                ex = trans.tile([128, 512], bf, tag="ex", bufs=6)
                                nc.scalar.activation(
                                    out=ex[:, qb:512], in_=pss[:, qb:512],
                                    func=AF.Exp,
                                )
                                if off >= 0:
                                    nc.vector.tensor_mul(
                                        ex[:, qb:qb + 128], ex[:, qb:qb + 128],
                                        mask,
                                    )
                                if j == 0:
                                    nc.vector.tensor_copy(acc, ex)
                                else:
                                    nc.vector.tensor_add(
                                        acc[:, qb:512], acc[:, qb:512],
                                        ex[:, qb:512],
                                    )
                                pending.append((j, ex, qb))
                            if pending:
                                consume_batch(pending, True)

                            psd = psumD.tile([128, 512], f32, tag="psd", bufs=1)
                            nc.tensor.matmul(
                                out=psd, lhsT=onesf, rhs=acc,
                                start=True, stop=True,
                            )
                            rdb = prdb.tile([128, 512], f32, tag="rdb", bufs=2)
                            nc.vector.reciprocal_approx_fast(out=rdb, in_=psd)
                            nc.vector.tensor_mul(attn_i[:, h, :], pso, rdb)
                        attn_tiles[i] = attn_i

                    def oproj_chunk(i):
                        attn_i = attn_tiles[i]
                        for m in range(TT):
                            psf = psumD.tile([128, 512], f32, tag="psm", bufs=2)
                            for kk in range(HPC):
                                nc.tensor.matmul(
                                    out=psf,
                                    lhsT=ow[:, kk, ts(m, 128)],
                                    rhs=attn_i[:, kk, :],
                                    start=(kk == 0),
                                    stop=(kk == HPC - 1),
                                )
                            ob = pob.tile([128, 512], bf, tag="ob", bufs=3)
                            if i == TQ - 1:
                                copy(eng(m), ob, psf)  # attention done: ACT free
                            else:
                                nc.vector.tensor_copy(ob, psf)
                            (nc.sync if m % 2 else nc.gpsimd).dma_start(
                                out=out_ap[ts(m, 128), ts(i, 512)], in_=ob
                            )

                    # big chunks first: the tail then drains on the
                    # smallest attention chunk + its o_proj
                    order = list(range(TQ))[::-1]
                    attention_chunk(order[0])
                    for k in range(1, TQ):
                        attention_chunk(order[k])
                        oproj_chunk(order[k - 1])
                    oproj_chunk(order[-1])

    nc.compile()
    return nc


def _tile_w(w):
    """[K, N] -> [N/128, 128, K/128, 128] so each col-block loads contiguously."""
    K, N = w.shape
    return np.ascontiguousarray(
        w.reshape(K // 128, 128, N // 128, 128).transpose(2, 1, 0, 3))


def _prep(inputs):
    x = np.asarray(inputs["hidden_states"], np.float32)
    qaw = np.asarray(inputs["q_a_w"], np.float32)
    qalw = np.asarray(inputs["q_a_ln_w"], np.float32)
    qbw = np.asarray(inputs["q_b_w"], np.float32)
    kvaw = np.asarray(inputs["kv_a_w"], np.float32)
    kvlw = np.asarray(inputs["kv_a_ln_w"], np.float32)
    kvbw = np.asarray(inputs["kv_b_w"], np.float32)
    ow = np.asarray(inputs["o_w"], np.float32)

    scale = 1.0 / np.sqrt(np.float32(HEAD_DIM))
    qbw_f = qbw * qalw[:, None] * scale
    wq_full = qaw @ qbw_f  # [HIDDEN, NUM_HEADS*HEAD_DIM] fp32 fold
    kvbw_f = (kvbw * kvlw[:, None]).astype(BF16)

    r = np.arange(128)[:, None]
    j = np.arange(128)[None, :]
    mask = np.where(j >= r, 1.0, 0.0).astype(BF16)
    ones128 = np.ones((128, 128), BF16)

    def lat_tiled(w):  # [KV_LORA, N] -> [128, NL, N] (p, kk, n)
        return np.ascontiguousarray(w.reshape(NL, 128, -1).transpose(1, 0, 2))

    in_maps = []
    for c in range(NCORES):
        b, g = c // 4, c % 4
        # folded W_q for this head group, pair layout [nope0|nope1|ropes]
        wq_g = wq_full[:, g * HPC * HEAD_DIM:(g + 1) * HPC * HEAD_DIM]
        pairs = []
        for pair in range(HPC // 2):
            h0, h1 = 2 * pair, 2 * pair + 1
            cols = np.concatenate([
                wq_g[:, h0 * HEAD_DIM:h0 * HEAD_DIM + QK_NOPE],
                wq_g[:, h1 * HEAD_DIM:h1 * HEAD_DIM + QK_NOPE],
                wq_g[:, h0 * HEAD_DIM + QK_NOPE:(h0 + 1) * HEAD_DIM],
                wq_g[:, h1 * HEAD_DIM + QK_NOPE:(h1 + 1) * HEAD_DIM],
            ], axis=1).astype(BF16)  # [HIDDEN, 384]
            pairs.append(cols.reshape(KC, 128, 384).transpose(1, 0, 2))
        wq_c = np.ascontiguousarray(np.stack(pairs))   # [2, 128, KC, 384]

        # norm-shard of q_a: 384 cols per core
        qnw = _tile_w(np.ascontiguousarray(
            qaw[:, g * 384:(g + 1) * 384]).astype(BF16))

        # kv_a: this core's 128-col latent shard + its 4 heads' rope cols
        kvaw_g = np.concatenate(
            [kvaw[:, g * 128:(g + 1) * 128],
             kvaw[:, KV_LORA + g * HPC * QK_ROPE:
                  KV_LORA + (g + 1) * HPC * QK_ROPE]], axis=1).astype(BF16)

        xt = np.ascontiguousarray(
            x[b].T.astype(BF16).reshape(KC, 128, TQ, 512).transpose(2, 1, 0, 3)
            .reshape(TQ, 128, KC * 512))
        in_maps.append({
            "x": xt,
            "qnw": qnw,
            "wq": wq_c,
            "kvaw": _tile_w(kvaw_g),
            "kvbw": lat_tiled(kvbw_f[:, g * HPC * 256:(g + 1) * HPC * 256]),
            "ow": np.ascontiguousarray(
                ow[g * HPC * V_HEAD:(g + 1) * HPC * V_HEAD]
                .astype(BF16).reshape(HPC, 128, HIDDEN).transpose(1, 0, 2)),
            "mask": mask,
            "ones128": ones128,
        })
    return in_maps


def _ensure_trace_shim():
    """This image lacks antenv.axon_hooks; synthesize it so a trace=True (or
    BASS_TRACE=1) invocation degrades gracefully instead of crashing."""
    import sys
    import types
    try:
        import antenv.axon_hooks  # noqa: F401
        return
    except Exception:
        pass
    try:
        import antenv
        import trn_agent_boot.trn_boot as tb
        hook = tb._ntff_profile_via_ctypes("/opt/axon/libaxon_pjrt.so")
        mod = types.ModuleType("antenv.axon_hooks")
        mod.get_axon_ntff_profile_hook = lambda: hook
        mod.set_axon_ntff_profile_hook = lambda h: None
        antenv.axon_hooks = mod
        sys.modules["antenv.axon_hooks"] = mod
        import concourse.bass_utils as bu
        bu.upload_artifacts = lambda tmpdir: tmpdir
    except Exception:
        pass


def kernel(**inputs):
    from concourse.bass_utils import run_bass_kernel_spmd

    _ensure_trace_shim()
    if "nc" not in _CACHE:
        _CACHE["nc"] = _build()
    nc = _CACHE["nc"]
    in_maps = _prep(inputs)
    try:
        res = run_bass_kernel_spmd(nc, in_maps, core_ids=list(range(NCORES)),
                                   **_CACHE.get("run_kwargs", {}))
    except Exception:
        # transient accelerator faults (e.g. NRT_EXEC_UNIT_UNRECOVERABLE) have
        # been observed after interrupted runs; one retry clears them
        import time
        time.sleep(2)
        res = run_bass_kernel_spmd(nc, in_maps, core_ids=list(range(NCORES)),
                                   **_CACHE.get("run_kwargs", {}))
    _CACHE["last_results"] = res
    out = np.zeros((B, T, HIDDEN), np.float32)
    for c in range(NCORES):
        out[c // 4] += np.asarray(res.results[c]["out"], np.float32).T
    return out
